# revision 38
# baseline (speedup 1.0000x reference)
"""Trainium2 Bass kernel: out = e + e @ B @ A^T  (low-rank residual update).

e: [4, 4096, 4096] f32, A/B: [4096, 16] f32.  Tolerance is rel_err < 2e-2,
which admits bf16 transfers: all DRAM I/O is bf16 (host casts), halving HBM
traffic vs the f32 kernel (32 MiB/core vs 64 -> ~94 us DMA floor vs ~188;
measured HW rel_err ~5.6e-3).

Layout trick: the host ships each core's row-shard TRANSPOSED and pre-tiled
(blk mode + pair_layout glay=4: [blk][kquad][p][j][r], d = (4*kq+j)*128+p,
blocks of B=1024 rows), so every DMA is a plain 2-D [128, 4B] transfer with
per-partition-contiguous 8 KiB descriptors (grouping via layout keeps the
APs 2-D, which walrus requires inside For_i), and the on-device PE-transpose +
PSUM->SBUF copy pipeline of the f32 kernel disappears (the d-contraction
wants d on partitions).  Per 1024-row block:
  s1:    for k in 32: load eT_k [128, 1024]; t[16, 1024] += b_k^T @ eT_k
  sweep: for c in 32: yT_c [128,1024] = at_c^T @ t (one LDW + 2 512-col mms)
         eT_c += yT_c in place (drain pattern "aad": 2/3 via ACT copy->bf16
         + DVE 2x add, 1/3 DVE direct 1x add), then store eT_c (deferred 6
         chunks so its adds-complete wait never blocks the SP stream).
PE sees two long same-weight-family phases per block (all-b_k then all-at_c):
HW-measured, interleaving the two doubles PE time (LDWEIGHTS serialization /
weight thrash: 72 -> 135 us), which the instruction-cost model does not
predict.  Host transposes the bf16 outputs back and casts f32.

HW (For_i repeat-loop slope, see bench.py; no NTFF hook under axon):
the shipped config (glay=4: quad-grouped DMAs, 8 KiB descriptors) beat its
pair-grouped bracket in 4 of 4 paired comparisons (98.4 vs 99.8/101.9 fast
window; 100.4/108.3 vs 111.1/110.5 slow window); pair-config measurements
spanned 93.5-101.9 us (mean 99).  Estimated shipped mean ~98 us vs a
94.3 us pure-DMA envelope for the same transfers and 202.5 us for the f32
baseline.  Grouping loads/stores via the DRAM layout and splitting the t
copy across ACT+DVE each measured ~2 us; ACT-ring stores lose ~16 us
(sequencer HOL blocking).
"""

import sys

sys.path.insert(0, "/opt/trn_rl_repo")

import numpy as np

import concourse.bass as bass
import concourse.mybir as mybir
import concourse.tile as tile


def _split_waits(nc, max_w=1):
    """The walrus in this container rejects instructions carrying more than
    ~2 sync-waits. Hoist extra waits onto same-engine NOPs placed directly
    before the offending instruction (engines execute their stream in
    order, so this is semantics-preserving)."""
    for f in nc.m.functions:
        for blk in f.blocks:
            insts = blk.instructions
            out = []
            changed = False
            for inst in insts:
                si = inst.sync_info
                if si is not None and si.on_wait and len(si.on_wait) > max_w:
                    waits = list(si.on_wait)
                    for j, w in enumerate(waits[max_w:]):
                        out.append(
                            mybir.InstNoOp(
                                name=f"{inst.name}-wsplit{j}",
                                sync_info=mybir.SyncInfo(on_wait=[w], on_update=[]),
                                bass_nofuse=True,
                                engine=inst.engine,
                            )
                        )
                    si.on_wait = waits[:max_w]
                    changed = True
                out.append(inst)
            if changed:
                blk.instructions = out


DIM = 4096
RANK = 16
N_CORES = 8
ROWS_TOTAL = 4 * 4096
ROWS_PER_CORE = ROWS_TOTAL // N_CORES  # 2048

BF16 = mybir.dt.bfloat16
F32 = mybir.dt.float32

KC = DIM // 128  # 32 k-tiles / d'-chunks


class _Cfg:
    def __init__(self, **kw):
        self.st_rows = kw.pop("st_rows", 256)
        self.add_mode = kw.pop("add_mode", "mix")  # direct | split | mix
        self.mix = kw.pop("mix", 3)
        self.grp = kw.pop("grp", 2)
        self.e_bufs = kw.pop("e_bufs", 10)
        self.y_bufs = kw.pop("y_bufs", 3)
        self.t_eng = kw.pop("t_eng", "scalar")
        self.store_eng = kw.pop("store_eng", "gpsimd")
        self.store_halves = kw.pop("store_halves", True)
        self.store_defer = kw.pop("store_defer", 0)  # groups to defer stores
        self.interleave = kw.pop("interleave", False)
        self.load_split = kw.pop("load_split", 1)  # DMAs per supertile load
        # none | dma (loads+stores only) | compute (no dma) | pe (no drain)
        # | noadd (PE+ACT copies, no DVE adds) | nostore (all but stores)
        # | ldonly (loads only) | s1 (loads+s1) | s2o (loads+s2-matmuls only)
        self.strip = kw.pop("strip", "none")
        self.mode = kw.pop("mode", "st")  # st (supertile) | blk (k-major blocks)
        # blk drain pattern, cycled per chunk: a=ACT copy + DVE add,
        # d=DVE direct add (1x PSUM), p=Pool(gpsimd) direct add,
        # v=DVE 2x copy + DVE 2x add
        self.drain_pat = kw.pop("drain_pat", "")
        self.ysb_bufs = kw.pop("ysb_bufs", 0)  # 0 = default
        self.e_split = kw.pop("e_split", 1)  # et sub-tiles per block (blk mode)
        # pair_layout: DRAM holds GROUPS of glay k/chunks side by side
        # ([.., p, glay*B]) so glay-chunk loads/stores are plain 2-D APs
        # with glay*2KiB descriptors
        self.pair_layout = kw.pop("pair_layout", False)
        self.glay = kw.pop("glay", 2)
        # t_split=2: copy t PSUM->SBUF in two half ops (ACT + DVE in
        # parallel) so sweep mm j=0 starts after only half the copy
        self.t_split = kw.pop("t_split", 1)
        self.ld_pair = kw.pop("ld_pair", 1)  # k-chunks per load DMA (blk mode)
        self.st_pair = kw.pop("st_pair", 1)  # chunks per store DMA (blk mode)
        self.blk_rows = kw.pop("blk_rows", 1024)
        self.rows_per_core = kw.pop("rows_per_core", ROWS_PER_CORE)
        assert not kw, f"unknown cfg keys: {kw}"
        assert self.rows_per_core % self.st_rows == 0 and KC % self.grp == 0
        self.n_st = self.rows_per_core // self.st_rows
        self.n_grp = KC // self.grp
        assert self.rows_per_core % self.blk_rows == 0
        self.n_blk = self.rows_per_core // self.blk_rows


def _emit_passes_blk(nc, tc, cfg, pools, b_sb, at_sb, et_ap, o_ap, n_passes):
    """Block-mode emission: rows in blocks of B (1024), k-major DRAM layout
    [blk][k][p][r] so loads/stores are [128, B] with 2*B-byte contiguous
    per-partition descriptors.  Per block: 32 (load k, s1 mm pair) -> t
    complete -> c-sweep of 32 chunks (LDW at_c once + 2 512-col matmuls ->
    [128, B] PSUM -> drain via mix of ACT-copy+DVE-add / DVE-direct-add,
    in-place into the e tile) with stores deferred a few chunks.  PE sees two
    long same-weight-family phases per block (s1 then sweep) instead of
    per-group weight thrash; every op is 2-4x bigger than st-mode."""
    cpool, epool, tpool, ypool, pst, psy = pools
    B, n_blk = cfg.blk_rows, cfg.n_blk
    NMM = max(B // 512, 1)  # 512-col PSUM-bank-sized matmuls per chunk
    ctx = {}
    pending = []
    cctr = [0]

    def flush(force=False):
        while pending and (force or pending[0][0] <= cctr[0]):
            pending.pop(0)[1]()

    ES = cfg.e_split
    EW = KC // ES  # k-chunks per et sub-tile

    def emit_loads_s1(h):
        hb = h % n_blk
        ets = [
            epool.tile([128, EW * B], BF16, name="et") for _ in range(ES)
        ]
        t_ps = pst.tile([RANK, B], F32, name="t_ps")
        ctx[h] = {"ets": ets, "t_ps": t_ps}
        for k in range(KC):
            p0 = (hb * KC + k) * 128
            et = ets[k // EW]
            kk = k % EW
            if cfg.strip != "compute" and cfg.pair_layout:
                g = cfg.glay
                if k % g == 0:  # one 2-D DMA covers k .. k+g-1
                    pp = (hb * (KC // g) + k // g) * 128
                    nc.sync.dma_start(
                        out=et[:, kk * B : (kk + g) * B],
                        in_=et_ap[pp : pp + 128, :],
                    )
            elif cfg.strip != "compute" and k % cfg.ld_pair == 0:
                n = cfg.ld_pair
                if n == 1:
                    nc.sync.dma_start(
                        out=et[:, kk * B : (kk + 1) * B], in_=et_ap[p0 : p0 + 128, :]
                    )
                else:
                    nc.sync.dma_start(
                        out=et[:, kk * B : (kk + n) * B].rearrange(
                            "p (c r) -> c p r", r=B
                        ),
                        in_=et_ap[p0 : p0 + n * 128, :].rearrange(
                            "(c p) r -> c p r", p=128
                        ),
                    )
            if cfg.strip in ("dma", "ldonly", "s2o"):
                continue
            for j in range(NMM):
                w = B // NMM
                nc.tensor.matmul(
                    t_ps[:, j * w : (j + 1) * w],
                    b_sb[:, k * RANK : (k + 1) * RANK],
                    et[:, kk * B + j * w : kk * B + (j + 1) * w],
                    start=(k == 0),
                    stop=(k == KC - 1),
                )
        if cfg.strip in ("dma", "ldonly", "s2o"):
            return
        t_sb = tpool.tile([RANK, B], BF16, name="t_sb")
        if cfg.t_split == 2:
            hw = B // 2
            nc.scalar.copy(t_sb[:, 0:hw], t_ps[:, 0:hw])
            nc.vector.tensor_copy(out=t_sb[:, hw:B], in_=t_ps[:, hw:B])
        elif cfg.t_eng == "vector":
            nc.vector.tensor_copy(out=t_sb, in_=t_ps)
        else:
            nc.scalar.copy(t_sb, t_ps)
        ctx[h]["t_sb"] = t_sb

    def emit_sweep(h):
        c = ctx[h]
        hb = h % n_blk
        for ch in range(KC):
            et = c["ets"][ch // EW]
            cc = ch % EW
            if cfg.strip not in ("dma", "ldonly", "s1"):
                t_sb = c.get("t_sb")
                yp = psy.tile([128, B], F32, name="yp")
                for j in range(NMM):
                    w = B // NMM
                    nc.tensor.matmul(
                        yp[:, j * w : (j + 1) * w],
                        at_sb[:, ch * 128 : (ch + 1) * 128],
                        t_sb[:, j * w : (j + 1) * w],
                        start=True,
                        stop=True,
                    )
                sl = slice(cc * B, (cc + 1) * B)
                if cfg.drain_pat:
                    kind = cfg.drain_pat[ch % len(cfg.drain_pat)]
                else:
                    kind = "a" if (
                        cfg.add_mode == "split"
                        or (cfg.add_mode == "mix" and (ch % 4) < cfg.mix)
                    ) else "d"
                if cfg.strip == "pe":
                    pass
                elif cfg.strip == "noadd":
                    if kind == "a":
                        ysb = ypool.tile([128, B], BF16, name="ysb")
                        nc.scalar.copy(ysb, yp)
                elif kind == "d":
                    nc.vector.tensor_add(out=et[:, sl], in0=et[:, sl], in1=yp)
                elif kind == "p":
                    nc.gpsimd.tensor_add(out=et[:, sl], in0=et[:, sl], in1=yp)
                elif kind == "v":
                    ysb = ypool.tile([128, B], BF16, name="ysb")
                    nc.vector.tensor_copy(out=ysb, in_=yp)
                    nc.vector.tensor_add(out=et[:, sl], in0=et[:, sl], in1=ysb)
                else:
                    ysb = ypool.tile([128, B], BF16, name="ysb")
                    nc.scalar.copy(ysb, yp)
                    nc.vector.tensor_add(out=et[:, sl], in0=et[:, sl], in1=ysb)

            def do_store(et=et, hb=hb, ch=ch, cc=cc, h=h, last=(ch == KC - 1)):
                if cfg.strip in ("none", "dma"):
                    if cfg.pair_layout:
                        g = cfg.glay
                        pp = (hb * (KC // g) + ch // g) * 128
                        getattr(nc, cfg.store_eng).dma_start(
                            out=o_ap[pp : pp + 128, :],
                            in_=et[:, (cc - g + 1) * B : (cc + 1) * B],
                        )
                        if last:
                            del ctx[h]
                        return
                    n = cfg.st_pair
                    c0 = cc - n + 1  # fires on the last chunk of the pair
                    p0 = (hb * KC + (ch - n + 1)) * 128
                    if n == 1:
                        nc.sync.dma_start(
                            out=o_ap[p0 : p0 + 128, :],
                            in_=et[:, cc * B : (cc + 1) * B],
                        )
                    else:
                        nc.sync.dma_start(
                            out=o_ap[p0 : p0 + n * 128, :].rearrange(
                                "(c p) r -> c p r", p=128
                            ),
                            in_=et[:, c0 * B : (cc + 1) * B].rearrange(
                                "p (c r) -> c p r", r=B
                            ),
                        )
                if last:
                    del ctx[h]

            sp = cfg.glay if cfg.pair_layout else cfg.st_pair
            if (ch + 1) % sp == 0:
                pending.append((cctr[0] + cfg.store_defer, do_store))
            cctr[0] += 1
            flush()

    total = n_passes * n_blk
    emit_loads_s1(0)
    for h in range(1, total + 1):
        if cfg.interleave:
            # sweep(h-1) first: its matmuls are ready (t done) while s1(h)
            # is load-paced; the in-order PE stream must not park on s1(h)
            emit_sweep(h - 1)
            if h < total:
                emit_loads_s1(h)
        else:
            if h < total:
                emit_loads_s1(h)
            emit_sweep(h - 1)
    flush(force=True)


def _emit_passes(nc, tc, cfg, pools, b_sb, at_sb, et_ap, o_ap, n_passes):
    """Emit n_passes full passes over the core's shard, software-pipelined.
    Stores are emitted cfg.store_defer groups after their half's adds so the
    adds-complete sem wait never blocks the issuing engine's stream."""
    cpool, epool, tpool, ypool, pst, psy = pools
    st_rows, grp, n_st, n_grp = cfg.st_rows, cfg.grp, cfg.n_st, cfg.n_grp
    t_const = None
    if cfg.strip == "s2o":
        t_const = cpool.tile([RANK, st_rows], BF16, name="t_const")
        nc.scalar.copy(t_const, b_sb[0:RANK, 0:st_rows])
    ctx = {}
    pending = []  # (due_gctr, emit_fn)

    def flush(gctr, force=False):
        while pending and (force or pending[0][0] <= gctr):
            pending.pop(0)[1]()

    def emit_load(st):
        p0 = (st % n_st) * 128
        et = epool.tile([128, KC * st_rows], BF16, name="et")
        if cfg.strip != "compute":
            ls = cfg.load_split
            w = KC * st_rows // ls
            for i in range(ls):
                nc.sync.dma_start(
                    out=et[:, i * w : (i + 1) * w],
                    in_=et_ap[p0 : p0 + 128, i * w : (i + 1) * w],
                )
        ctx[st] = {"et": et}

    def emit_s1_macro(st, m, kk):
        if cfg.strip in ("dma", "ldonly", "s2o"):
            return
        c = ctx[st]
        if m == 0:
            c["t_ps"] = pst.tile([RANK, st_rows], F32, name="t_ps")
        t_ps, et = c["t_ps"], c["et"]
        for k in range(m * kk, (m + 1) * kk):
            nc.tensor.matmul(
                t_ps,
                b_sb[:, k * RANK : (k + 1) * RANK],
                et[:, k * st_rows : (k + 1) * st_rows],
                start=(k == 0),
                stop=(k == KC - 1),
            )
        if (m + 1) * kk == KC:
            t_sb = tpool.tile([RANK, st_rows], BF16, name="t_sb")
            if cfg.t_eng == "vector":
                nc.vector.tensor_copy(out=t_sb, in_=t_ps)
            else:
                nc.scalar.copy(t_sb, t_ps)
            c["t_sb"] = t_sb

    def emit_s1(st):
        emit_s1_macro(st, 0, KC)

    def emit_s2_grp(st, g, gctr):
        c = ctx[st]
        if cfg.strip in ("dma", "ldonly", "s1"):
            _emit_store_maybe(st, g, gctr, c)
            return
        if cfg.strip == "s2o":
            yp = psy.tile([128, grp * st_rows], F32, name="yp")
            for j in range(grp):
                ch = g * grp + j
                nc.tensor.matmul(
                    yp[:, j * st_rows : (j + 1) * st_rows],
                    at_sb[:, ch * 128 : (ch + 1) * 128],
                    t_const,
                    start=True,
                    stop=True,
                )
            _emit_store_maybe(st, g, gctr, c)
            return
        strip_store = cfg.strip in ("compute", "pe", "noadd", "nostore")
        et, t_sb = c["et"], c["t_sb"]
        yp = psy.tile([128, grp * st_rows], F32, name="yp")
        for j in range(grp):
            ch = g * grp + j
            nc.tensor.matmul(
                yp[:, j * st_rows : (j + 1) * st_rows],
                at_sb[:, ch * 128 : (ch + 1) * 128],
                t_sb,
                start=True,
                stop=True,
            )
        sl = slice(g * grp * st_rows, (g + 1) * grp * st_rows)
        use_split = cfg.add_mode == "split" or (
            cfg.add_mode == "mix" and (g % 4) < cfg.mix
        )
        if cfg.strip == "pe":
            pass
        elif cfg.strip == "noadd":
            if use_split:
                ysb = ypool.tile([128, grp * st_rows], BF16, name="ysb")
                nc.scalar.copy(ysb, yp)
        elif not use_split:
            nc.vector.tensor_add(out=et[:, sl], in0=et[:, sl], in1=yp)
        else:
            ysb = ypool.tile([128, grp * st_rows], BF16, name="ysb")
            nc.scalar.copy(ysb, yp)
            nc.vector.tensor_add(out=et[:, sl], in0=et[:, sl], in1=ysb)
        _emit_store_maybe(st, g, gctr, c)

    def _emit_store_maybe(st, g, gctr, c):
        et = c["et"]
        p0 = (st % n_st) * 128
        n_half = n_grp // 2 if cfg.store_halves else n_grp
        if (g + 1) % n_half == 0:
            h = (g + 1) // n_half - 1
            cs = slice(h * n_half * grp * st_rows, (h + 1) * n_half * grp * st_rows)

            def do_store(et=et, p0=p0, cs=cs, st=st, last=(g == n_grp - 1)):
                if cfg.strip not in ("compute", "pe", "noadd", "nostore",
                                     "ldonly", "s1", "s2o"):
                    getattr(nc, cfg.store_eng).dma_start(
                        out=o_ap[p0 : p0 + 128, cs], in_=et[:, cs]
                    )
                if last:
                    del ctx[st]

            pending.append((gctr + cfg.store_defer, do_store))

    gctr = 0
    total_st = n_passes * n_st
    if not cfg.interleave:
        for st in range(total_st):
            emit_load(st)
            emit_s1(st)
            for g in range(n_grp):
                flush(gctr)
                emit_s2_grp(st, g, gctr)
                gctr += 1
        flush(gctr, force=True)
        return

    # software-pipelined emission: s2(st-1) groups interspersed into
    # s1(st)'s k-loop so the in-order PE/ACT/DVE streams alternate between
    # the previous supertile's PSUM drain and the next one's accumulation
    kk = KC // n_grp  # s1 k-chunks per interleave step
    emit_load(0)
    emit_s1(0)
    for st in range(1, total_st + 1):
        if st < total_st:
            emit_load(st)
        for m in range(n_grp):
            if st < total_st:
                emit_s1_macro(st, m, kk)
            flush(gctr)
            emit_s2_grp(st - 1, m, gctr)
            gctr += 1
    flush(gctr, force=True)


def _build_pools(tc, cfg):
    return (
        tc.tile_pool(name="const", bufs=1),
        tc.tile_pool(name="epool", bufs=cfg.e_bufs),
        tc.tile_pool(name="tpool", bufs=2),
        tc.tile_pool(name="ypool", bufs=cfg.ysb_bufs or max(2 * cfg.grp, 4)),
        tc.tile_pool(name="pst", bufs=2 if cfg.mode != "blk" else 1, space="PSUM"),
        tc.tile_pool(name="psy", bufs=cfg.y_bufs, space="PSUM"),
    )


def _load_consts(nc, cpool, b_in, at_in):
    # const loads go on the ACT HWDGE ring so the first e-load (SP ring)
    # is not queued behind them during the fill
    b_sb = cpool.tile([128, KC * RANK], BF16, name="b_sb")
    nc.scalar.dma_start(
        out=b_sb.rearrange("p (k j) -> p k j", j=RANK),
        in_=b_in.ap().rearrange("(k p) j -> p k j", p=128),
    )
    at_sb = cpool.tile([RANK, DIM], BF16, name="at_sb")
    nc.scalar.dma_start(out=at_sb, in_=at_in.ap()[:, :])
    return b_sb, at_sb


def _dram_shape(cfg):
    if cfg.mode == "blk":
        if cfg.pair_layout:
            g = cfg.glay
            return [cfg.n_blk * (KC // g) * 128, g * cfg.blk_rows]
        return [cfg.n_blk * KC * 128, cfg.blk_rows]
    return [cfg.n_st * 128, KC * cfg.st_rows]


def build_nc(reps=1, split_waits=True, **kw):
    cfg = _Cfg(**kw)
    nc = bass.Bass("TRN2", target_bir_lowering=False, debug=False)
    shp = _dram_shape(cfg)
    et_in = nc.dram_tensor("et_in", shp, BF16, kind="ExternalInput")
    b_in = nc.dram_tensor("b_in", [DIM, RANK], BF16, kind="ExternalInput")
    at_in = nc.dram_tensor("at_in", [RANK, DIM], BF16, kind="ExternalInput")
    out_d = nc.dram_tensor("out_d", shp, BF16, kind="ExternalOutput")

    with tile.TileContext(nc) as tc:
        ps = _build_pools(tc, cfg)
        with ps[0] as cpool, ps[1] as epool, ps[2] as tpool, ps[3] as ypool, \
             ps[4] as pst, ps[5] as psy:
            b_sb, at_sb = _load_consts(nc, cpool, b_in, at_in)
            emit = _emit_passes_blk if cfg.mode == "blk" else _emit_passes
            emit(
                nc, tc, cfg, (cpool, epool, tpool, ypool, pst, psy),
                b_sb, at_sb, et_in.ap(), out_d.ap(), n_passes=reps,
            )
    if split_waits:
        _split_waits(nc)
    return nc


def build_timing_nc(loop_n=1024, body_passes=2, split_waits=True, **kw):
    """Timing-only variant: same per-pass instruction stream as build_nc, but
    e/out live in Internal DRAM (no host transfer; contents are don't-care)
    and a For_i hardware loop repeats body_passes passes loop_n times.  A
    tiny external out ("tick") keeps the NEFF I/O contract valid.  Wall-clock
    deltas over loop_n give per-pass device time with the axon tunnel noise
    amortized away.  NOTE: SWDGE (gpsimd) DMA inside For_i breaks walrus
    codegen ("ISA wrong length"), so store_eng must be sync/scalar here."""
    cfg = _Cfg(**kw)
    assert cfg.store_eng in ("sync", "scalar"), "SWDGE store breaks in For_i"
    nc = bass.Bass("TRN2", target_bir_lowering=False, debug=False)
    shp = _dram_shape(cfg)
    et_in = nc.dram_tensor("et_in", shp, BF16, kind="Internal")
    b_in = nc.dram_tensor("b_in", [DIM, RANK], BF16, kind="ExternalInput")
    at_in = nc.dram_tensor("at_in", [RANK, DIM], BF16, kind="ExternalInput")
    out_d = nc.dram_tensor("out_d", shp, BF16, kind="Internal")
    tick = nc.dram_tensor("tick", [1, RANK], F32, kind="ExternalOutput")

    with tile.TileContext(nc) as tc:
        ps = _build_pools(tc, cfg)
        with ps[0] as cpool, ps[1] as epool, ps[2] as tpool, ps[3] as ypool, \
             ps[4] as pst, ps[5] as psy:
            b_sb, at_sb = _load_consts(nc, cpool, b_in, at_in)
            emit = _emit_passes_blk if cfg.mode == "blk" else _emit_passes
            with tc.For_i(0, loop_n):
                emit(
                    nc, tc, cfg, (cpool, epool, tpool, ypool, pst, psy),
                    b_sb, at_sb, et_in.ap(), out_d.ap(), n_passes=body_passes,
                )
            tick_sb = cpool.tile([1, RANK], F32, name="tick_sb")
            nc.vector.tensor_copy(out=tick_sb, in_=at_sb[0:1, 0:RANK])
            nc.sync.dma_start(out=tick.ap()[:, :], in_=tick_sb)
    if split_waits:
        _split_waits(nc)
    return nc


_NC_CACHE = {}

ST_ROWS = 256
BUILD_KW = dict(
    mode="blk", blk_rows=1024, e_bufs=2, y_bufs=3, drain_pat="aad",
    t_eng="scalar", store_eng="sync", store_defer=6, pair_layout=True,
    t_split=2, glay=4,
)


def _get_nc(rows_per_core=ROWS_PER_CORE):
    key = rows_per_core
    if key not in _NC_CACHE:
        kw = dict(BUILD_KW)
        kw["rows_per_core"] = rows_per_core
        _NC_CACHE[key] = build_nc(**kw)
    return _NC_CACHE[key]


def _pack(e_shard_f32, cfg_kw, bf16):
    # st mode: [rows, DIM] f32 -> [n_st*128, KC*st_rows] tiled [st][p][k][r]
    # blk mode: -> [n_blk*KC*128, B] tiled [blk][k][p][r]   (d = k*128 + p)
    # blk+pair: -> [n_blk*KC/2*128, 2B] tiled [blk][kp][p][j][r] (k = 2kp+j)
    rows = e_shard_f32.shape[0]
    if cfg_kw.get("mode") == "blk":
        B = cfg_kw["blk_rows"]
        n_blk = rows // B
        if cfg_kw.get("pair_layout"):
            g = cfg_kw.get("glay", 2)
            a = e_shard_f32.reshape(n_blk, B, KC // g, g, 128)  # [blk,r,kg,j,p]
            a = a.transpose(0, 2, 4, 3, 1).astype(bf16)  # [blk, kg, p, j, r]
            return np.ascontiguousarray(a.reshape(n_blk * (KC // g) * 128, g * B))
        a = e_shard_f32.reshape(n_blk, B, KC, 128)  # [blk, r, k, p]
        a = a.transpose(0, 2, 3, 1).astype(bf16)  # [blk, k, p, r]
        return np.ascontiguousarray(a.reshape(n_blk * KC * 128, B))
    st_rows = cfg_kw["st_rows"]
    n_st = rows // st_rows
    a = e_shard_f32.reshape(n_st, st_rows, KC, 128)  # [st, r, k, p]
    a = a.transpose(0, 3, 2, 1).astype(bf16)  # [st, p, k, r]
    return np.ascontiguousarray(a.reshape(n_st * 128, -1))


def _unpack(o_tiled, cfg_kw):
    if cfg_kw.get("mode") == "blk":
        B = cfg_kw["blk_rows"]
        if cfg_kw.get("pair_layout"):
            g = cfg_kw.get("glay", 2)
            n_blk = o_tiled.shape[0] // ((KC // g) * 128)
            a = np.asarray(o_tiled).reshape(n_blk, KC // g, 128, g, B)
            a = a.astype(np.float32)  # [blk, kg, p, j, r]
            # -> [blk, r, kg, j, p] -> [rows, DIM]
            return a.transpose(0, 4, 1, 3, 2).reshape(n_blk * B, DIM)
        n_blk = o_tiled.shape[0] // (KC * 128)
        a = np.asarray(o_tiled).reshape(n_blk, KC, 128, B).astype(np.float32)
        return a.transpose(0, 3, 1, 2).reshape(n_blk * B, DIM)
    st_rows = cfg_kw["st_rows"]
    n_st = o_tiled.shape[0] // 128
    a = np.asarray(o_tiled).reshape(n_st, 128, KC, st_rows).astype(np.float32)
    return a.transpose(0, 3, 2, 1).reshape(n_st * st_rows, DIM)


def kernel(e, A, B):
    from concourse.bass_utils import run_bass_kernel_spmd
    import ml_dtypes

    bf16 = ml_dtypes.bfloat16
    e = np.asarray(e, dtype=np.float32)
    A = np.asarray(A, dtype=np.float32)
    B = np.asarray(B, dtype=np.float32)
    batch, seq, dim = e.shape
    rows = batch * seq
    e2 = e.reshape(rows, dim)
    rpc = rows // N_CORES

    b_bf = B.astype(bf16)
    at_bf = np.ascontiguousarray(A.T).astype(bf16)
    in_maps = [
        {
            "et_in": _pack(e2[i * rpc : (i + 1) * rpc], BUILD_KW, bf16),
            "b_in": b_bf,
            "at_in": at_bf,
        }
        for i in range(N_CORES)
    ]
    nc = _get_nc(rpc)
    res = run_bass_kernel_spmd(nc, in_maps, core_ids=list(range(N_CORES)))
    out = np.empty((rows, dim), dtype=np.float32)
    for i in range(N_CORES):
        out[i * rpc : (i + 1) * rpc] = _unpack(res.results[i]["out_d"], BUILD_KW)
    return out.reshape(batch, seq, dim)
